# revision 32
# baseline (speedup 1.0000x reference)
"""Causal self-attention (B=2, T=2048, C=1024, H=16) on 8 TRN2 NeuronCores.

Sharding: core c -> batch b = c//4, heads 4*(c%4) .. 4*(c%4)+3.
Each core computes q,k,v for its 4 heads (column-parallel qkv), causal
attention, and a partial output projection over its heads' rows of
w_proj (row-parallel). Host sums the 4 partials per batch and adds
b_proj (with the v-bias folded in: P@(V+bv)/l = P@V/l + bv, so
b_eff = b_proj + bv @ w_proj is added host-side for free).

Precision plan (gate is rel_err < 2e-2; fp8 producers were tried and
measured 3-6e-2 -- softmax amplifies score perturbations -- so all
matmul operands are bf16; PSUM accumulation fp32; fp32r streams
~1.35-1.44x slower than bf16 at K=128, SBUF-read-bandwidth bound).
Output HBM tensor is bf16, upcast + summed across cores in fp32 host-
side (halves the tail output DMA).

Device schedule (per core, SPMD), designed so the PE never idles:
  - Inputs DMA'd in consumption order; xT packed as (tb, p, kc, 512)
    so each 512-token chunk is one contiguous descriptor per partition;
    descriptor generation split across both HWDGE queues (Sync + ACT)
    to halve the DMA head.
  - qT,kT in [cols, tokens] layout; scores built transposed
    (S^T[j,i] = k_j . q_i). Head pairs at partition offsets 0/64 issue
    their K=64 score matmuls back-to-back on disjoint PE row groups.
  - exp on ACT only (one merged instruction per j-tile). Causal mask:
    off-diagonal blocks skipped, matmul N-ranges below the diagonal,
    triu multiply (DVE) on the 128x128 diagonal blocks.
  - attention inner loop is software-pipelined: scores(jt) are issued
    before PV(jt-1), so the PE runs one iteration ahead of ACT's exp.
  - P@V accumulated as out^T[d,i] with V stationary; a ones-column in
    V yields the softmax denominator as PSUM row 64 for free.
  - normalization: reciprocal_approx_fast -> gpsimd partition_broadcast
    -> DVE multiply into attT; emission deferred past filler units
    (att_block returns a closure) so the PE-order never blocks on it,
    and placed after the qk bias-adds in DVE order.
  - proj PSUM eviction on ACT (phase-disjoint with exp: exp runs
    during attention windows, evictions during filler windows).
  - PSUM pools stream-separated (scores 2x2 banks, PV accum 2x1,
    producer/proj 2x1 = 8 banks).
"""
import numpy as np

import concourse.bacc as bacc
import concourse.bass as bass
import concourse.mybir as mybir
import concourse.tile as tile
from concourse.bass_utils import run_bass_kernel_spmd

F32 = mybir.dt.float32
BF16 = mybir.dt.bfloat16
AF = mybir.ActivationFunctionType

B, T, C = 2, 2048, 1024
H, DH = 16, 64
HPC = 4                    # heads per core
VCOLS = HPC * DH           # 256
KC = C // 128              # 8 contraction chunks
TT = T // 128              # 16 token tiles
NB = T // 512              # 4 i-blocks


def build_nc():
    nc = bacc.Bacc("TRN2", target_bir_lowering=False, debug=False, num_devices=8)

    xT_d = nc.dram_tensor("xT", (NB, 128, KC, 512), BF16, kind="ExternalInput")
    wqk_d = nc.dram_tensor("wqk4", (4, 128, KC, 128), BF16, kind="ExternalInput")
    bqk_d = nc.dram_tensor("bqk", (128, 4), F32, kind="ExternalInput")
    wv_d = nc.dram_tensor("wv4", (128, KC, VCOLS), BF16, kind="ExternalInput")
    wp_d = nc.dram_tensor("wp4", (128, 2, C), BF16, kind="ExternalInput")
    triu_d = nc.dram_tensor("triu", (128, 128), BF16, kind="ExternalInput")
    ones_d = nc.dram_tensor("ones64", (128, 64), BF16, kind="ExternalInput")
    out_d = nc.dram_tensor("out", (T, C), BF16, kind="ExternalOutput")

    with tile.TileContext(nc) as tc:
        with (
            tc.tile_pool(name="persist", bufs=1) as pp,
            tc.tile_pool(name="work", bufs=4) as pw,
            tc.tile_pool(name="nrm", bufs=3) as pn,
            tc.tile_pool(name="osb", bufs=3) as po,
            tc.tile_pool(name="ps_s", bufs=2, space="PSUM") as ps_s,
            tc.tile_pool(name="ps_oa", bufs=2, space="PSUM") as ps_oa,
            tc.tile_pool(name="ps_w", bufs=2, space="PSUM") as ps_w,
        ):
            # ---- persistent tiles ----
            triu = pp.tile([128, 128], BF16, tag="triu")
            ones64 = pp.tile([128, 64], BF16, tag="ones64")
            bqk_sb = pp.tile([128, 4], F32, tag="bqk")
            wqk_sb = pp.tile([128, 4, KC, 128], BF16, tag="wqk")
            wv_sb = pp.tile([128, KC, VCOLS], BF16, tag="wv")
            wp_sb = pp.tile([128, 2, C], BF16, tag="wp")
            xT_sb = pp.tile([128, NB, KC, 512], BF16, tag="xT")
            qkT = pp.tile([128, 4, T], BF16, tag="qkT")
            v_sb = pp.tile([128, TT, HPC, DH + 1], BF16, tag="v_sb")
            attT = pp.tile([128, 2, T], BF16, tag="attT")

            # ---- DMAs in consumption/priority order; issue split across
            # the two HWDGE queues (Sync gets the first-needed tensors,
            # ACT the consts + late x chunks) ----
            # DMA descriptors fan out across all rings, so every issued
            # transfer steals bandwidth from the first-needed bytes: emit
            # only the soon-needed tensors here (3MB, ~7us) and the rest
            # later in the program (xT1 after att(0,0), xT2+wp at bi=1,
            # xT3 at bi=2).
            # (HWDGE queues allow ~4 outstanding DMAs each; rings serve all
            # in-flight transfers together, so the first wave is kept small
            # and kc0 of x gets its own transfer for the earliest matmul)
            xT_ap = xT_d.ap().rearrange("tb p kc t -> p tb kc t")
            nc.scalar.dma_start(xT_sb[:, 0, 0:1], xT_ap[:, 0, 0:1])
            nc.sync.dma_start(wqk_sb[:, 0], wqk_d.ap()[0])
            nc.sync.dma_start(xT_sb[:, 0, 1:3], xT_ap[:, 0, 1:3])
            nc.sync.dma_start(xT_sb[:, 0, 3:8], xT_ap[:, 0, 3:8])
            nc.scalar.dma_start(bqk_sb[:], bqk_d.ap())
            nc.scalar.dma_start(triu[:], triu_d.ap())
            nc.scalar.dma_start(ones64[:], ones_d.ap())
            nc.sync.dma_start(wqk_sb[:, 2], wqk_d.ap()[2])
            nc.sync.dma_start(wv_sb[:], wv_d.ap())
            for ct in (1, 3):
                nc.scalar.dma_start(wqk_sb[:, ct], wqk_d.ap()[ct])

            # ACT exp-table pre-warm during the DMA head
            warm = pw.tile([1, 8], F32, tag="warm")
            nc.scalar.activation(warm[:], wqk_sb[0:1, 0, 0, 0:8], AF.Exp)

            # ones column of V (softmax denominator) written once
            nc.vector.tensor_copy(
                v_sb[:, :, :, DH],
                ones64[:].rearrange("p (a b) -> p a b", a=TT),
            )

            # ---- work units ----
            def qk_unit(ct, tb):
                ps = ps_w.tile([128, 512], F32, tag="w", name=f"qk{ct}_{tb}")
                for kc in range(KC):
                    nc.tensor.matmul(
                        ps[:],
                        wqk_sb[:, ct, kc, :],
                        xT_sb[:, tb, kc, :],
                        start=(kc == 0),
                        stop=(kc == KC - 1),
                    )
                nc.vector.tensor_scalar_add(
                    qkT[:, ct, tb * 512 : (tb + 1) * 512], ps[:], bqk_sb[:, ct : ct + 1]
                )

            def v_unit(tt):
                ps = ps_w.tile([128, 512], F32, tag="w", name=f"v{tt}")
                for kc in range(KC):
                    nc.tensor.matmul(
                        ps[:, 0:VCOLS],
                        xT_sb[:, tt // 4, kc, (tt % 4) * 128 : (tt % 4 + 1) * 128],
                        wv_sb[:, kc, :],
                        start=(kc == 0),
                        stop=(kc == KC - 1),
                    )
                nc.vector.tensor_copy(
                    v_sb[:, tt, :, 0:DH],
                    ps[:, 0:VCOLS].rearrange("p (h d) -> p h d", h=HPC),
                )

            def att_block(bi, g, fillers=None):
                """Emits scores/exp/PV; returns a closure emitting the
                normalization (call it after some filler matmuls so the
                PE-order never blocks on the DVE/gpsimd chain). `fillers`
                maps pipeline position jt -> list of filler-unit callables
                emitted after scores(jt), before PV(jt-1): position 1
                gives ACT's exp pipeline runway so PV(0) never stalls;
                mid-block positions absorb ACT's per-tile deficit in the
                long blocks."""
                ioff = bi * 512
                njt = 4 * bi + 4
                qT = [qkT[0:64, g, :], qkT[64:128, g, :]]
                kT = [qkT[0:64, 2 + g, :], qkT[64:128, 2 + g, :]]
                oa = [
                    ps_oa.tile([DH + 1, 512], F32, tag="oa", name=f"oa{bi}_{g}_{u}")
                    for u in range(2)
                ]
                exq = [None] * njt
                # software pipeline: scores(jt) issued before PV(jt-1)
                for jt in range(njt + 1):
                    if jt < njt:
                        d = jt - 4 * bi
                        so = d * 128 if d > 0 else 0
                        # u=1's cols start at 512 (not 512+so) so the score
                        # region [so : 1024-so] is contiguous and one merged
                        # exp instruction covers exactly the valid columns
                        ub = [so, 512]
                        ps = ps_s.tile([128, 1024], F32, tag="s")
                        ex = pw.tile([128, 1024], BF16, tag="exp")
                        for u in range(2):
                            nc.tensor.matmul(
                                ps[:, ub[u] : ub[u] + 512 - so],
                                kT[u][:, jt * 128 : (jt + 1) * 128],
                                qT[u][:, ioff + so : ioff + 512],
                                start=True,
                                stop=True,
                            )
                        nc.scalar.activation(
                            ex[:, so : 1024 - so], ps[:, so : 1024 - so], AF.Exp
                        )
                        if d >= 0:
                            for u in range(2):
                                nc.vector.tensor_mul(
                                    ex[:, ub[u] : ub[u] + 128],
                                    ex[:, ub[u] : ub[u] + 128],
                                    triu[:],
                                )
                        exq[jt] = (ex, so, ub)
                    if fillers is not None and jt in fillers:
                        for f in fillers[jt]:
                            f()
                    if jt > 0:
                        ex, so, ub = exq[jt - 1]
                        for u in range(2):
                            nc.tensor.matmul(
                                oa[u][:, so:512],
                                v_sb[:, jt - 1, 2 * g + u, :],
                                ex[:, ub[u] : ub[u] + 512 - so],
                                start=(jt - 1 == 0),
                                stop=(jt - 1 == njt - 1),
                            )

                def norm():
                    # normalization for the head pair (reciprocal_approx_fast
                    # needs an SBUF partition-0 input on HW, hence lrow copy)
                    tail = bi == NB - 1 and g == 1
                    for u in range(2):
                        lrow = pn.tile(
                            [1, 512], F32, tag="lrow", name=f"lw{bi}_{g}_{u}"
                        )
                        if tail:
                            nc.scalar.copy(lrow[:], oa[u][DH : DH + 1, :])
                        else:
                            nc.vector.tensor_copy(lrow[:], oa[u][DH : DH + 1, :])
                        rst = pn.tile(
                            [1, 512], F32, tag="rst", name=f"rs{bi}_{g}_{u}"
                        )
                        nc.vector.reciprocal_approx_fast(rst[:], lrow[:])
                        rb = pn.tile([DH, 512], F32, tag="rb")
                        nc.gpsimd.partition_broadcast(rb[:], rst[:])
                        if tail:
                            # split so the first tail proj units' attT deps
                            # resolve half a chain earlier
                            for k in range(2):
                                nc.vector.tensor_mul(
                                    attT[
                                        64 * u : 64 * u + 64,
                                        g,
                                        ioff + 256 * k : ioff + 256 * (k + 1),
                                    ],
                                    oa[u][0:DH, 256 * k : 256 * (k + 1)],
                                    rb[:, 256 * k : 256 * (k + 1)],
                                )
                        else:
                            nc.vector.tensor_mul(
                                attT[64 * u : 64 * u + 64, g, ioff : ioff + 512],
                                oa[u][0:DH, :],
                                rb[:],
                            )

                return norm

            def proj_unit(tt, half, tail=False):
                ps = ps_w.tile([128, 512], F32, tag="w", name=f"p{tt}_{half}")
                for kc2 in range(2):
                    nc.tensor.matmul(
                        ps[:],
                        attT[:, kc2, tt * 128 : (tt + 1) * 128],
                        wp_sb[:, kc2, half * 512 : (half + 1) * 512],
                        start=(kc2 == 0),
                        stop=(kc2 == 1),
                    )
                osb = po.tile([128, 512], BF16, tag="osb")
                if tail:
                    nc.scalar.copy(osb[:], ps[:])  # ACT is idle at the tail
                else:
                    nc.vector.tensor_copy(osb[:], ps[:])
                nc.sync.dma_start(
                    out_d.ap()[tt * 128 : (tt + 1) * 128, half * 512 : (half + 1) * 512],
                    osb[:],
                )

            # ---- weave: attention paced, producer/proj as PE filler;
            # qk units for the next head pair are the in-block fillers;
            # norm closures emitted one block late ----
            # (fillers must not feed the enclosing block's own scores(0):
            # att(bi,0) uses ct0/ct2 only, so qk(1,bi) is safe; att(bi,1)
            # for bi>=1 only needs qk(3,bi)'s tokens from step 4bi >= 4;
            # att(0,1) would deadlock on qk(3,0), so it gets v_unit(4).)
            qk_unit(0, 0)
            qk_unit(2, 0)
            for tt in range(4):
                v_unit(tt)
            P = lambda tt, half: (lambda: proj_unit(tt, half))
            n0 = att_block(0, 0, fillers={1: [lambda: qk_unit(1, 0)]})
            nc.sync.dma_start(xT_sb[:, 1], xT_ap[:, 1])
            qk_unit(3, 0)
            n0()
            n1 = att_block(0, 1, fillers={1: [lambda: v_unit(4)]})
            # bi = 1
            nc.sync.dma_start(xT_sb[:, 2], xT_ap[:, 2])
            nc.scalar.dma_start(wp_sb[:], wp_d.ap())
            qk_unit(0, 1)
            qk_unit(2, 1)
            n1()
            for tt in range(5, 8):
                v_unit(tt)
            for tt in range(0, 4):
                proj_unit(tt, 0)
                proj_unit(tt, 1)
            n0 = att_block(1, 0, fillers={1: [lambda: qk_unit(1, 1)]})
            n0()
            n1 = att_block(1, 1, fillers={1: [lambda: qk_unit(3, 1)]})
            # bi = 2
            nc.sync.dma_start(xT_sb[:, 3], xT_ap[:, 3])
            qk_unit(0, 2)
            qk_unit(2, 2)
            n1()
            for tt in range(8, 12):
                v_unit(tt)
            for tt in (4, 5, 7):
                proj_unit(tt, 0)
                proj_unit(tt, 1)
            n0 = att_block(2, 0, fillers={1: [lambda: qk_unit(1, 2)], 7: [P(6, 0)]})
            n0()
            n1 = att_block(2, 1, fillers={1: [lambda: qk_unit(3, 2)], 7: [P(6, 1)]})
            # bi = 3
            qk_unit(0, 3)
            qk_unit(2, 3)
            n1()
            for tt in range(12, 16):
                v_unit(tt)
            for tt in (8, 9):
                proj_unit(tt, 0)
                proj_unit(tt, 1)
            n0 = att_block(
                3, 0, fillers={1: [lambda: qk_unit(1, 3)], 7: [P(10, 0)]}
            )
            n0()
            n1 = att_block(3, 1, fillers={1: [lambda: qk_unit(3, 3)]})
            # PE work reserved for the final norm-chain window (the manual
            # wait keeps the scheduler from hoisting it into earlier
            # bubbles; covering this window also keeps HAM warm for the
            # tail projections)
            with tc.tile_wait_until(0.128):
                proj_unit(10, 1)
                proj_unit(11, 0)
                proj_unit(11, 1)
            n1()
            for tt in range(12, 16):
                proj_unit(tt, 0, tail=True)
                proj_unit(tt, 1, tail=True)

    nc.compile()
    return nc


def make_core_inputs(x, w_qkv, b_qkv, w_proj, b_proj):
    """Per-core input maps (host-side sharding)."""
    bf = mybir.dt.np(BF16)
    x = np.asarray(x, dtype=np.float32)
    w_qkv = np.asarray(w_qkv, dtype=np.float32)
    b_qkv = np.asarray(b_qkv, dtype=np.float32)
    w_proj = np.asarray(w_proj, dtype=np.float32)

    consts = {
        "triu": np.triu(np.ones((128, 128), dtype=np.float32)).astype(bf),
        "ones64": np.ones((128, 64), dtype=np.float32).astype(bf),
    }
    in_maps = []
    for c in range(8):
        b = c // 4
        heads = [4 * (c % 4) + i for i in range(HPC)]
        qcols = np.concatenate([np.arange(64 * h, 64 * h + 64) for h in heads])
        wq = w_qkv[:, qcols] * 0.125
        bq = b_qkv[qcols] * 0.125
        wk = w_qkv[:, C + qcols]
        bk = b_qkv[C + qcols]
        wv = w_qkv[:, 2 * C + qcols]
        wqk = np.concatenate([wq, wk], axis=1)          # [C, 512]
        # wqk4[ct, p, kc, m] = wqk[kc*128+p, ct*128+m]
        wqk4 = np.ascontiguousarray(
            wqk.reshape(KC, 128, 4, 128).transpose(2, 1, 0, 3)
        ).astype(bf)
        bqk = np.concatenate([bq, bk]).reshape(4, 128).T.copy()
        # wv4[p, kc, m] = wv[kc*128+p, m]
        wv4 = np.ascontiguousarray(
            wv.reshape(KC, 128, VCOLS).transpose(1, 0, 2)
        ).astype(bf)
        # wp4[p, kc2, n] = w_proj[qcols[kc2*128+p], n]
        wp4 = np.ascontiguousarray(
            w_proj[qcols, :].reshape(2, 128, C).transpose(1, 0, 2)
        ).astype(bf)
        # xT6[tb, p, kc, t] = x[b][tb*512+t, kc*128+p]
        xT6 = np.ascontiguousarray(
            x[b].reshape(NB, 512, KC, 128).transpose(0, 3, 2, 1)
        ).astype(bf)
        in_maps.append({
            "xT": xT6,
            "wqk4": wqk4,
            "bqk": bqk,
            "wv4": wv4,
            "wp4": wp4,
            **consts,
        })
    return in_maps


_NC_CACHE = []


def kernel(x, w_qkv, b_qkv, w_proj, b_proj):
    if not _NC_CACHE:
        _NC_CACHE.append(build_nc())
    nc = _NC_CACHE[0]
    in_maps = make_core_inputs(x, w_qkv, b_qkv, w_proj, b_proj)
    res = run_bass_kernel_spmd(nc, in_maps, list(range(8)))
    w_proj = np.asarray(w_proj, dtype=np.float64)
    bv = np.asarray(b_qkv, dtype=np.float64)[2 * C : 3 * C]
    b_eff = (np.asarray(b_proj, dtype=np.float64) + bv @ w_proj).astype(np.float32)
    out = np.empty((B, T, C), dtype=np.float32)
    for b in range(B):
        acc = res.results[4 * b]["out"].astype(np.float32)
        for c in range(4 * b + 1, 4 * b + 4):
            acc = acc + res.results[c]["out"].astype(np.float32)
        out[b] = acc + b_eff
    return out


# revision 33
# speedup vs baseline: 1.0064x; 1.0064x over previous
"""Causal self-attention (B=2, T=2048, C=1024, H=16) on 8 TRN2 NeuronCores.

Sharding: core c -> batch b = c//4, heads 4*(c%4) .. 4*(c%4)+3.
Each core computes q,k,v for its 4 heads (column-parallel qkv), causal
attention, and a partial output projection over its heads' rows of
w_proj (row-parallel). Host sums the 4 partials per batch and adds
b_proj (with the v-bias folded in: P@(V+bv)/l = P@V/l + bv, so
b_eff = b_proj + bv @ w_proj is added host-side for free).

Precision plan (gate is rel_err < 2e-2; fp8 producers were tried and
measured 3-6e-2 -- softmax amplifies score perturbations -- so all
matmul operands are bf16; PSUM accumulation fp32; fp32r streams
~1.35-1.44x slower than bf16 at K=128, SBUF-read-bandwidth bound).
Output HBM tensor is bf16, upcast + summed across cores in fp32 host-
side (halves the tail output DMA).

Device schedule (per core, SPMD), designed so the PE never idles:
  - Inputs DMA'd in consumption order; xT packed as (tb, p, kc, 512)
    so each 512-token chunk is one contiguous descriptor per partition;
    descriptor generation split across both HWDGE queues (Sync + ACT)
    to halve the DMA head.
  - qT,kT in [cols, tokens] layout; scores built transposed
    (S^T[j,i] = k_j . q_i). Head pairs at partition offsets 0/64 issue
    their K=64 score matmuls back-to-back on disjoint PE row groups.
  - exp on ACT only (one merged instruction per j-tile). Causal mask:
    off-diagonal blocks skipped, matmul N-ranges below the diagonal,
    triu multiply (DVE) on the 128x128 diagonal blocks.
  - attention inner loop is software-pipelined: scores(jt) are issued
    before PV(jt-1), so the PE runs one iteration ahead of ACT's exp.
  - P@V accumulated as out^T[d,i] with V stationary; a ones-column in
    V yields the softmax denominator as PSUM row 64 for free.
  - normalization: reciprocal_approx_fast -> gpsimd partition_broadcast
    -> DVE multiply into attT; emission deferred past filler units
    (att_block returns a closure) so the PE-order never blocks on it,
    and placed after the qk bias-adds in DVE order.
  - proj PSUM eviction on ACT (phase-disjoint with exp: exp runs
    during attention windows, evictions during filler windows).
  - PSUM pools stream-separated (scores 2x2 banks, PV accum 2x1,
    producer/proj 2x1 = 8 banks).
"""
import numpy as np

import concourse.bacc as bacc
import concourse.bass as bass
import concourse.mybir as mybir
import concourse.tile as tile
from concourse.bass_utils import run_bass_kernel_spmd

F32 = mybir.dt.float32
BF16 = mybir.dt.bfloat16
AF = mybir.ActivationFunctionType

B, T, C = 2, 2048, 1024
H, DH = 16, 64
HPC = 4                    # heads per core
VCOLS = HPC * DH           # 256
KC = C // 128              # 8 contraction chunks
TT = T // 128              # 16 token tiles
NB = T // 512              # 4 i-blocks


def build_nc():
    nc = bacc.Bacc("TRN2", target_bir_lowering=False, debug=False, num_devices=8)

    xT_d = nc.dram_tensor("xT", (NB, 128, KC, 512), BF16, kind="ExternalInput")
    wqk_d = nc.dram_tensor("wqk4", (4, 128, KC, 128), BF16, kind="ExternalInput")
    bqk_d = nc.dram_tensor("bqk", (128, 4), F32, kind="ExternalInput")
    wv_d = nc.dram_tensor("wv4", (128, KC, VCOLS), BF16, kind="ExternalInput")
    wp_d = nc.dram_tensor("wp4", (128, 2, C), BF16, kind="ExternalInput")
    triu_d = nc.dram_tensor("triu", (128, 128), BF16, kind="ExternalInput")
    ones_d = nc.dram_tensor("ones64", (128, 64), BF16, kind="ExternalInput")
    out_d = nc.dram_tensor("out", (T, C), BF16, kind="ExternalOutput")

    with tile.TileContext(nc) as tc:
        with (
            tc.tile_pool(name="persist", bufs=1) as pp,
            tc.tile_pool(name="work", bufs=4) as pw,
            tc.tile_pool(name="nrm", bufs=3) as pn,
            tc.tile_pool(name="osb", bufs=3) as po,
            tc.tile_pool(name="ps_s", bufs=2, space="PSUM") as ps_s,
            tc.tile_pool(name="ps_oa", bufs=2, space="PSUM") as ps_oa,
            tc.tile_pool(name="ps_w", bufs=2, space="PSUM") as ps_w,
        ):
            # ---- persistent tiles ----
            triu = pp.tile([128, 128], BF16, tag="triu")
            ones64 = pp.tile([128, 64], BF16, tag="ones64")
            bqk_sb = pp.tile([128, 4], F32, tag="bqk")
            wqk_sb = pp.tile([128, 4, KC, 128], BF16, tag="wqk")
            wv_sb = pp.tile([128, KC, VCOLS], BF16, tag="wv")
            wp_sb = pp.tile([128, 2, C], BF16, tag="wp")
            xT_sb = pp.tile([128, NB, KC, 512], BF16, tag="xT")
            qkT = pp.tile([128, 4, T], BF16, tag="qkT")
            v_sb = pp.tile([128, TT, HPC, DH + 1], BF16, tag="v_sb")
            attT = pp.tile([128, 2, T], BF16, tag="attT")

            # ---- DMAs in consumption/priority order; issue split across
            # the two HWDGE queues (Sync gets the first-needed tensors,
            # ACT the consts + late x chunks) ----
            # DMA descriptors fan out across all rings, so every issued
            # transfer steals bandwidth from the first-needed bytes: emit
            # only the soon-needed tensors here (3MB, ~7us) and the rest
            # later in the program (xT1 after att(0,0), xT2+wp at bi=1,
            # xT3 at bi=2).
            # (HWDGE queues allow ~4 outstanding DMAs each; rings serve all
            # in-flight transfers together, so the first wave is kept small
            # and kc0 of x gets its own transfer for the earliest matmul)
            xT_ap = xT_d.ap().rearrange("tb p kc t -> p tb kc t")
            nc.scalar.dma_start(xT_sb[:, 0, 0:1], xT_ap[:, 0, 0:1])
            nc.sync.dma_start(wqk_sb[:, 0], wqk_d.ap()[0])
            nc.sync.dma_start(xT_sb[:, 0, 1:3], xT_ap[:, 0, 1:3])
            nc.sync.dma_start(xT_sb[:, 0, 3:8], xT_ap[:, 0, 3:8])
            nc.scalar.dma_start(bqk_sb[:], bqk_d.ap())
            nc.scalar.dma_start(triu[:], triu_d.ap())
            nc.scalar.dma_start(ones64[:], ones_d.ap())
            nc.sync.dma_start(wqk_sb[:, 2], wqk_d.ap()[2])
            nc.sync.dma_start(wv_sb[:], wv_d.ap())
            for ct in (1, 3):
                nc.scalar.dma_start(wqk_sb[:, ct], wqk_d.ap()[ct])

            # ACT exp-table pre-warm during the DMA head
            warm = pw.tile([1, 8], F32, tag="warm")
            nc.scalar.activation(warm[:], wqk_sb[0:1, 0, 0, 0:8], AF.Exp)

            # ones column of V (softmax denominator) written once
            nc.vector.tensor_copy(
                v_sb[:, :, :, DH],
                ones64[:].rearrange("p (a b) -> p a b", a=TT),
            )

            # ---- work units ----
            def qk_unit(ct, tb):
                ps = ps_w.tile([128, 512], F32, tag="w", name=f"qk{ct}_{tb}")
                for kc in range(KC):
                    nc.tensor.matmul(
                        ps[:],
                        wqk_sb[:, ct, kc, :],
                        xT_sb[:, tb, kc, :],
                        start=(kc == 0),
                        stop=(kc == KC - 1),
                    )
                nc.vector.tensor_scalar_add(
                    qkT[:, ct, tb * 512 : (tb + 1) * 512], ps[:], bqk_sb[:, ct : ct + 1]
                )

            def v_unit(tt):
                ps = ps_w.tile([128, 512], F32, tag="w", name=f"v{tt}")
                for kc in range(KC):
                    nc.tensor.matmul(
                        ps[:, 0:VCOLS],
                        xT_sb[:, tt // 4, kc, (tt % 4) * 128 : (tt % 4 + 1) * 128],
                        wv_sb[:, kc, :],
                        start=(kc == 0),
                        stop=(kc == KC - 1),
                    )
                nc.vector.tensor_copy(
                    v_sb[:, tt, :, 0:DH],
                    ps[:, 0:VCOLS].rearrange("p (h d) -> p h d", h=HPC),
                )

            def att_block(bi, g, fillers=None):
                """Emits scores/exp/PV; returns a closure emitting the
                normalization (call it after some filler matmuls so the
                PE-order never blocks on the DVE/gpsimd chain). `fillers`
                maps pipeline position jt -> list of filler-unit callables
                emitted after scores(jt), before PV(jt-1): position 1
                gives ACT's exp pipeline runway so PV(0) never stalls;
                mid-block positions absorb ACT's per-tile deficit in the
                long blocks."""
                ioff = bi * 512
                njt = 4 * bi + 4
                qT = [qkT[0:64, g, :], qkT[64:128, g, :]]
                kT = [qkT[0:64, 2 + g, :], qkT[64:128, 2 + g, :]]
                oa = [
                    ps_oa.tile([DH + 1, 512], F32, tag="oa", name=f"oa{bi}_{g}_{u}")
                    for u in range(2)
                ]
                exq = [None] * njt
                # software pipeline: scores(jt) issued before PV(jt-1)
                for jt in range(njt + 1):
                    if jt < njt:
                        d = jt - 4 * bi
                        so = d * 128 if d > 0 else 0
                        # u=1's cols start at 512 (not 512+so) so the score
                        # region [so : 1024-so] is contiguous and one merged
                        # exp instruction covers exactly the valid columns
                        ub = [so, 512]
                        ps = ps_s.tile([128, 1024], F32, tag="s")
                        ex = pw.tile([128, 1024], BF16, tag="exp")
                        for u in range(2):
                            nc.tensor.matmul(
                                ps[:, ub[u] : ub[u] + 512 - so],
                                kT[u][:, jt * 128 : (jt + 1) * 128],
                                qT[u][:, ioff + so : ioff + 512],
                                start=True,
                                stop=True,
                            )
                        nc.scalar.activation(
                            ex[:, so : 1024 - so], ps[:, so : 1024 - so], AF.Exp
                        )
                        if d >= 0:
                            for u in range(2):
                                nc.vector.tensor_mul(
                                    ex[:, ub[u] : ub[u] + 128],
                                    ex[:, ub[u] : ub[u] + 128],
                                    triu[:],
                                )
                        exq[jt] = (ex, so, ub)
                    if fillers is not None and jt in fillers:
                        for f in fillers[jt]:
                            f()
                    if jt > 0:
                        ex, so, ub = exq[jt - 1]
                        for u in range(2):
                            nc.tensor.matmul(
                                oa[u][:, so:512],
                                v_sb[:, jt - 1, 2 * g + u, :],
                                ex[:, ub[u] : ub[u] + 512 - so],
                                start=(jt - 1 == 0),
                                stop=(jt - 1 == njt - 1),
                            )

                def norm():
                    # normalization for the head pair (reciprocal_approx_fast
                    # needs an SBUF partition-0 input on HW, hence lrow copy)
                    tail = bi == NB - 1 and g == 1
                    for u in range(2):
                        lrow = pn.tile(
                            [1, 512], F32, tag="lrow", name=f"lw{bi}_{g}_{u}"
                        )
                        if tail:
                            nc.scalar.copy(lrow[:], oa[u][DH : DH + 1, :])
                        else:
                            nc.vector.tensor_copy(lrow[:], oa[u][DH : DH + 1, :])
                        rst = pn.tile(
                            [1, 512], F32, tag="rst", name=f"rs{bi}_{g}_{u}"
                        )
                        nc.vector.reciprocal_approx_fast(rst[:], lrow[:])
                        rb = pn.tile([DH, 512], F32, tag="rb")
                        nc.gpsimd.partition_broadcast(rb[:], rst[:])
                        if tail:
                            # split so the first tail proj units' attT deps
                            # resolve half a chain earlier
                            for k in range(2):
                                nc.vector.tensor_mul(
                                    attT[
                                        64 * u : 64 * u + 64,
                                        g,
                                        ioff + 256 * k : ioff + 256 * (k + 1),
                                    ],
                                    oa[u][0:DH, 256 * k : 256 * (k + 1)],
                                    rb[:, 256 * k : 256 * (k + 1)],
                                )
                        else:
                            nc.vector.tensor_mul(
                                attT[64 * u : 64 * u + 64, g, ioff : ioff + 512],
                                oa[u][0:DH, :],
                                rb[:],
                            )

                return norm

            def proj_unit(tt, half, tail=False):
                ps = ps_w.tile([128, 512], F32, tag="w", name=f"p{tt}_{half}")
                for kc2 in range(2):
                    nc.tensor.matmul(
                        ps[:],
                        attT[:, kc2, tt * 128 : (tt + 1) * 128],
                        wp_sb[:, kc2, half * 512 : (half + 1) * 512],
                        start=(kc2 == 0),
                        stop=(kc2 == 1),
                    )
                osb = po.tile([128, 512], BF16, tag="osb")
                if tail:
                    nc.scalar.copy(osb[:], ps[:])  # ACT is idle at the tail
                else:
                    nc.vector.tensor_copy(osb[:], ps[:])
                nc.sync.dma_start(
                    out_d.ap()[tt * 128 : (tt + 1) * 128, half * 512 : (half + 1) * 512],
                    osb[:],
                )

            # ---- weave: attention paced, producer/proj as PE filler;
            # qk units for the next head pair are the in-block fillers;
            # norm closures emitted one block late ----
            # (fillers must not feed the enclosing block's own scores(0):
            # att(bi,0) uses ct0/ct2 only, so qk(1,bi) is safe; att(bi,1)
            # for bi>=1 only needs qk(3,bi)'s tokens from step 4bi >= 4;
            # att(0,1) would deadlock on qk(3,0), so it gets v_unit(4).)
            qk_unit(0, 0)
            qk_unit(2, 0)
            for tt in range(4):
                v_unit(tt)
            P = lambda tt, half: (lambda: proj_unit(tt, half))
            n0 = att_block(0, 0, fillers={1: [lambda: qk_unit(1, 0)]})
            nc.sync.dma_start(xT_sb[:, 1], xT_ap[:, 1])
            qk_unit(3, 0)
            n0()
            n1 = att_block(0, 1, fillers={1: [lambda: v_unit(4)]})
            # bi = 1
            nc.sync.dma_start(xT_sb[:, 2], xT_ap[:, 2])
            nc.scalar.dma_start(wp_sb[:], wp_d.ap())
            qk_unit(0, 1)
            qk_unit(2, 1)
            n1()
            for tt in range(5, 8):
                v_unit(tt)
            for tt in range(0, 4):
                proj_unit(tt, 0)
                proj_unit(tt, 1)
            n0 = att_block(1, 0, fillers={1: [lambda: qk_unit(1, 1)]})
            n0()
            n1 = att_block(1, 1, fillers={1: [lambda: qk_unit(3, 1)]})
            # bi = 2
            nc.sync.dma_start(xT_sb[:, 3], xT_ap[:, 3])
            qk_unit(0, 2)
            qk_unit(2, 2)
            n1()
            for tt in range(8, 12):
                v_unit(tt)
            for tt in (4, 5, 7):
                proj_unit(tt, 0)
                proj_unit(tt, 1)
            n0 = att_block(2, 0, fillers={1: [lambda: qk_unit(1, 2)], 7: [P(6, 0)]})
            n0()
            n1 = att_block(2, 1, fillers={1: [lambda: qk_unit(3, 2)], 7: [P(6, 1)]})
            # bi = 3
            qk_unit(0, 3)
            qk_unit(2, 3)
            n1()
            for tt in range(12, 16):
                v_unit(tt)
            for tt in (8, 9):
                proj_unit(tt, 0)
                proj_unit(tt, 1)
            n0 = att_block(
                3, 0, fillers={1: [lambda: qk_unit(1, 3)], 7: [P(10, 0)], 12: [P(10, 1)]}
            )
            n0()
            n1 = att_block(3, 1, fillers={1: [lambda: qk_unit(3, 3)], 7: [P(11, 0)]})
            proj_unit(11, 1)
            n1()
            for tt in range(12, 16):
                proj_unit(tt, 0, tail=True)
                proj_unit(tt, 1, tail=True)

    nc.compile()
    return nc


def make_core_inputs(x, w_qkv, b_qkv, w_proj, b_proj):
    """Per-core input maps (host-side sharding)."""
    bf = mybir.dt.np(BF16)
    x = np.asarray(x, dtype=np.float32)
    w_qkv = np.asarray(w_qkv, dtype=np.float32)
    b_qkv = np.asarray(b_qkv, dtype=np.float32)
    w_proj = np.asarray(w_proj, dtype=np.float32)

    consts = {
        "triu": np.triu(np.ones((128, 128), dtype=np.float32)).astype(bf),
        "ones64": np.ones((128, 64), dtype=np.float32).astype(bf),
    }
    in_maps = []
    for c in range(8):
        b = c // 4
        heads = [4 * (c % 4) + i for i in range(HPC)]
        qcols = np.concatenate([np.arange(64 * h, 64 * h + 64) for h in heads])
        wq = w_qkv[:, qcols] * 0.125
        bq = b_qkv[qcols] * 0.125
        wk = w_qkv[:, C + qcols]
        bk = b_qkv[C + qcols]
        wv = w_qkv[:, 2 * C + qcols]
        wqk = np.concatenate([wq, wk], axis=1)          # [C, 512]
        # wqk4[ct, p, kc, m] = wqk[kc*128+p, ct*128+m]
        wqk4 = np.ascontiguousarray(
            wqk.reshape(KC, 128, 4, 128).transpose(2, 1, 0, 3)
        ).astype(bf)
        bqk = np.concatenate([bq, bk]).reshape(4, 128).T.copy()
        # wv4[p, kc, m] = wv[kc*128+p, m]
        wv4 = np.ascontiguousarray(
            wv.reshape(KC, 128, VCOLS).transpose(1, 0, 2)
        ).astype(bf)
        # wp4[p, kc2, n] = w_proj[qcols[kc2*128+p], n]
        wp4 = np.ascontiguousarray(
            w_proj[qcols, :].reshape(2, 128, C).transpose(1, 0, 2)
        ).astype(bf)
        # xT6[tb, p, kc, t] = x[b][tb*512+t, kc*128+p]
        xT6 = np.ascontiguousarray(
            x[b].reshape(NB, 512, KC, 128).transpose(0, 3, 2, 1)
        ).astype(bf)
        in_maps.append({
            "xT": xT6,
            "wqk4": wqk4,
            "bqk": bqk,
            "wv4": wv4,
            "wp4": wp4,
            **consts,
        })
    return in_maps


_NC_CACHE = []


def kernel(x, w_qkv, b_qkv, w_proj, b_proj):
    if not _NC_CACHE:
        _NC_CACHE.append(build_nc())
    nc = _NC_CACHE[0]
    in_maps = make_core_inputs(x, w_qkv, b_qkv, w_proj, b_proj)
    res = run_bass_kernel_spmd(nc, in_maps, list(range(8)))
    w_proj = np.asarray(w_proj, dtype=np.float64)
    bv = np.asarray(b_qkv, dtype=np.float64)[2 * C : 3 * C]
    b_eff = (np.asarray(b_proj, dtype=np.float64) + bv @ w_proj).astype(np.float32)
    out = np.empty((B, T, C), dtype=np.float32)
    for b in range(B):
        acc = res.results[4 * b]["out"].astype(np.float32)
        for c in range(4 * b + 1, 4 * b + 4):
            acc = acc + res.results[c]["out"].astype(np.float32)
        out[b] = acc + b_eff
    return out


# revision 34
# speedup vs baseline: 1.0269x; 1.0204x over previous
"""Causal self-attention (B=2, T=2048, C=1024, H=16) on 8 TRN2 NeuronCores.

Sharding: core c -> batch b = c//4, heads 4*(c%4) .. 4*(c%4)+3.
Each core computes q,k,v for its 4 heads (column-parallel qkv), causal
attention, and a partial output projection over its heads' rows of
w_proj (row-parallel). Host sums the 4 partials per batch and adds
b_proj (with the v-bias folded in: P@(V+bv)/l = P@V/l + bv, so
b_eff = b_proj + bv @ w_proj is added host-side for free).

Precision plan (gate is rel_err < 2e-2; fp8 producers were tried and
measured 3-6e-2 -- softmax amplifies score perturbations -- so all
matmul operands are bf16; PSUM accumulation fp32; fp32r streams
~1.35-1.44x slower than bf16 at K=128, SBUF-read-bandwidth bound).
Output HBM tensor is bf16, upcast + summed across cores in fp32 host-
side (halves the tail output DMA).

Device schedule (per core, SPMD), designed so the PE never idles:
  - Inputs DMA'd in consumption order; xT packed as (tb, p, kc, 512)
    so each 512-token chunk is one contiguous descriptor per partition;
    descriptor generation split across both HWDGE queues (Sync + ACT)
    to halve the DMA head.
  - qT,kT in [cols, tokens] layout; scores built transposed
    (S^T[j,i] = k_j . q_i). Head pairs at partition offsets 0/64 issue
    their K=64 score matmuls back-to-back on disjoint PE row groups.
  - exp on ACT only (one merged instruction per j-tile). Causal mask:
    off-diagonal blocks skipped, matmul N-ranges below the diagonal,
    triu multiply (DVE) on the 128x128 diagonal blocks.
  - attention inner loop is software-pipelined: scores(jt) are issued
    before PV(jt-1), so the PE runs one iteration ahead of ACT's exp.
  - P@V accumulated as out^T[d,i] with V stationary; a ones-column in
    V yields the softmax denominator as PSUM row 64 for free.
  - normalization: reciprocal_approx_fast -> gpsimd partition_broadcast
    -> DVE multiply into attT; emission deferred past filler units
    (att_block returns a closure) so the PE-order never blocks on it,
    and placed after the qk bias-adds in DVE order.
  - proj PSUM eviction on ACT (phase-disjoint with exp: exp runs
    during attention windows, evictions during filler windows).
  - PSUM pools stream-separated (scores 2x2 banks, PV accum 2x1,
    producer/proj 2x1 = 8 banks).
"""
import numpy as np

import concourse.bacc as bacc
import concourse.bass as bass
import concourse.mybir as mybir
import concourse.tile as tile
from concourse.bass_utils import run_bass_kernel_spmd

F32 = mybir.dt.float32
BF16 = mybir.dt.bfloat16
AF = mybir.ActivationFunctionType

B, T, C = 2, 2048, 1024
H, DH = 16, 64
HPC = 4                    # heads per core
VCOLS = HPC * DH           # 256
KC = C // 128              # 8 contraction chunks
TT = T // 128              # 16 token tiles
NB = T // 512              # 4 i-blocks


def build_nc():
    nc = bacc.Bacc("TRN2", target_bir_lowering=False, debug=False, num_devices=8)

    xT_d = nc.dram_tensor("xT", (NB, 128, KC, 512), BF16, kind="ExternalInput")
    wqk_d = nc.dram_tensor("wqk4", (4, 128, KC, 128), BF16, kind="ExternalInput")
    bqk_d = nc.dram_tensor("bqk", (128, 4), F32, kind="ExternalInput")
    wv_d = nc.dram_tensor("wv4", (128, KC, VCOLS), BF16, kind="ExternalInput")
    wp_d = nc.dram_tensor("wp4", (128, 2, C), BF16, kind="ExternalInput")
    triu_d = nc.dram_tensor("triu", (128, 128), BF16, kind="ExternalInput")
    ones_d = nc.dram_tensor("ones64", (128, 64), BF16, kind="ExternalInput")
    out_d = nc.dram_tensor("out", (T, C), BF16, kind="ExternalOutput")

    with tile.TileContext(nc) as tc:
        with (
            tc.tile_pool(name="persist", bufs=1) as pp,
            tc.tile_pool(name="work", bufs=4) as pw,
            tc.tile_pool(name="nrm", bufs=3) as pn,
            tc.tile_pool(name="osb", bufs=3) as po,
            tc.tile_pool(name="ps_s", bufs=2, space="PSUM") as ps_s,
            tc.tile_pool(name="ps_oa", bufs=2, space="PSUM") as ps_oa,
            tc.tile_pool(name="ps_w", bufs=2, space="PSUM") as ps_w,
        ):
            # ---- persistent tiles ----
            triu = pp.tile([128, 128], BF16, tag="triu")
            ones64 = pp.tile([128, 64], BF16, tag="ones64")
            bqk_sb = pp.tile([128, 4], F32, tag="bqk")
            wqk_sb = pp.tile([128, 4, KC, 128], BF16, tag="wqk")
            wv_sb = pp.tile([128, KC, VCOLS], BF16, tag="wv")
            wp_sb = pp.tile([128, 2, C], BF16, tag="wp")
            xT_sb = pp.tile([128, NB, KC, 512], BF16, tag="xT")
            qkT = pp.tile([128, 4, T], BF16, tag="qkT")
            v_sb = pp.tile([128, TT, HPC, DH + 1], BF16, tag="v_sb")
            attT = pp.tile([128, 2, T], BF16, tag="attT")

            # ---- DMAs in consumption/priority order; issue split across
            # the two HWDGE queues (Sync gets the first-needed tensors,
            # ACT the consts + late x chunks) ----
            # DMA descriptors fan out across all rings, so every issued
            # transfer steals bandwidth from the first-needed bytes: emit
            # only the soon-needed tensors here (3MB, ~7us) and the rest
            # later in the program (xT1 after att(0,0), xT2+wp at bi=1,
            # xT3 at bi=2).
            # (HWDGE queues allow ~4 outstanding DMAs each; rings serve all
            # in-flight transfers together, so the first wave is kept small
            # and kc0 of x gets its own transfer for the earliest matmul)
            xT_ap = xT_d.ap().rearrange("tb p kc t -> p tb kc t")
            nc.sync.dma_start(wqk_sb[:, 0, 0:1], wqk_d.ap()[0][:, 0:1])
            nc.scalar.dma_start(xT_sb[:, 0, 0:1], xT_ap[:, 0, 0:1])
            nc.sync.dma_start(wqk_sb[:, 0, 1:8], wqk_d.ap()[0][:, 1:8])
            nc.sync.dma_start(xT_sb[:, 0, 1:3], xT_ap[:, 0, 1:3])
            nc.sync.dma_start(wqk_sb[:, 2], wqk_d.ap()[2])
            nc.scalar.dma_start(bqk_sb[:], bqk_d.ap())
            nc.scalar.dma_start(triu[:], triu_d.ap())
            nc.scalar.dma_start(ones64[:], ones_d.ap())
            nc.sync.dma_start(xT_sb[:, 0, 3:8], xT_ap[:, 0, 3:8])
            nc.sync.dma_start(wv_sb[:], wv_d.ap())
            for ct in (1, 3):
                nc.scalar.dma_start(wqk_sb[:, ct], wqk_d.ap()[ct])

            # ACT exp-table pre-warm during the DMA head
            warm = pw.tile([1, 8], F32, tag="warm")
            nc.scalar.activation(warm[:], wqk_sb[0:1, 0, 0, 0:8], AF.Exp)

            # ones column of V (softmax denominator) written once
            nc.vector.tensor_copy(
                v_sb[:, :, :, DH],
                ones64[:].rearrange("p (a b) -> p a b", a=TT),
            )

            # ---- work units ----
            def qk_unit(ct, tb):
                ps = ps_w.tile([128, 512], F32, tag="w", name=f"qk{ct}_{tb}")
                for kc in range(KC):
                    nc.tensor.matmul(
                        ps[:],
                        wqk_sb[:, ct, kc, :],
                        xT_sb[:, tb, kc, :],
                        start=(kc == 0),
                        stop=(kc == KC - 1),
                    )
                nc.vector.tensor_scalar_add(
                    qkT[:, ct, tb * 512 : (tb + 1) * 512], ps[:], bqk_sb[:, ct : ct + 1]
                )

            def v_unit(tt):
                ps = ps_w.tile([128, 512], F32, tag="w", name=f"v{tt}")
                for kc in range(KC):
                    nc.tensor.matmul(
                        ps[:, 0:VCOLS],
                        xT_sb[:, tt // 4, kc, (tt % 4) * 128 : (tt % 4 + 1) * 128],
                        wv_sb[:, kc, :],
                        start=(kc == 0),
                        stop=(kc == KC - 1),
                    )
                nc.vector.tensor_copy(
                    v_sb[:, tt, :, 0:DH],
                    ps[:, 0:VCOLS].rearrange("p (h d) -> p h d", h=HPC),
                )

            def att_block(bi, g, fillers=None):
                """Emits scores/exp/PV; returns a closure emitting the
                normalization (call it after some filler matmuls so the
                PE-order never blocks on the DVE/gpsimd chain). `fillers`
                maps pipeline position jt -> list of filler-unit callables
                emitted after scores(jt), before PV(jt-1): position 1
                gives ACT's exp pipeline runway so PV(0) never stalls;
                mid-block positions absorb ACT's per-tile deficit in the
                long blocks."""
                ioff = bi * 512
                njt = 4 * bi + 4
                qT = [qkT[0:64, g, :], qkT[64:128, g, :]]
                kT = [qkT[0:64, 2 + g, :], qkT[64:128, 2 + g, :]]
                oa = [
                    ps_oa.tile([DH + 1, 512], F32, tag="oa", name=f"oa{bi}_{g}_{u}")
                    for u in range(2)
                ]
                exq = [None] * njt
                # software pipeline: scores(jt) issued before PV(jt-1)
                for jt in range(njt + 1):
                    if jt < njt:
                        d = jt - 4 * bi
                        so = d * 128 if d > 0 else 0
                        # u=1's cols start at 512 (not 512+so) so the score
                        # region [so : 1024-so] is contiguous and one merged
                        # exp instruction covers exactly the valid columns
                        ub = [so, 512]
                        ps = ps_s.tile([128, 1024], F32, tag="s")
                        ex = pw.tile([128, 1024], BF16, tag="exp")
                        for u in range(2):
                            nc.tensor.matmul(
                                ps[:, ub[u] : ub[u] + 512 - so],
                                kT[u][:, jt * 128 : (jt + 1) * 128],
                                qT[u][:, ioff + so : ioff + 512],
                                start=True,
                                stop=True,
                            )
                        nc.scalar.activation(
                            ex[:, so : 1024 - so], ps[:, so : 1024 - so], AF.Exp
                        )
                        if d >= 0:
                            for u in range(2):
                                nc.vector.tensor_mul(
                                    ex[:, ub[u] : ub[u] + 128],
                                    ex[:, ub[u] : ub[u] + 128],
                                    triu[:],
                                )
                        exq[jt] = (ex, so, ub)
                    if fillers is not None and jt in fillers:
                        for f in fillers[jt]:
                            f()
                    if jt > 0:
                        ex, so, ub = exq[jt - 1]
                        for u in range(2):
                            nc.tensor.matmul(
                                oa[u][:, so:512],
                                v_sb[:, jt - 1, 2 * g + u, :],
                                ex[:, ub[u] : ub[u] + 512 - so],
                                start=(jt - 1 == 0),
                                stop=(jt - 1 == njt - 1),
                            )

                def norm():
                    # normalization for the head pair (reciprocal_approx_fast
                    # needs an SBUF partition-0 input on HW, hence lrow copy)
                    tail = bi == NB - 1 and g == 1
                    for u in range(2):
                        lrow = pn.tile(
                            [1, 512], F32, tag="lrow", name=f"lw{bi}_{g}_{u}"
                        )
                        if tail:
                            nc.scalar.copy(lrow[:], oa[u][DH : DH + 1, :])
                        else:
                            nc.vector.tensor_copy(lrow[:], oa[u][DH : DH + 1, :])
                        rst = pn.tile(
                            [1, 512], F32, tag="rst", name=f"rs{bi}_{g}_{u}"
                        )
                        nc.vector.reciprocal_approx_fast(rst[:], lrow[:])
                        rb = pn.tile([DH, 512], F32, tag="rb")
                        nc.gpsimd.partition_broadcast(rb[:], rst[:])
                        if tail:
                            # split so the first tail proj units' attT deps
                            # resolve half a chain earlier
                            for k in range(2):
                                nc.vector.tensor_mul(
                                    attT[
                                        64 * u : 64 * u + 64,
                                        g,
                                        ioff + 256 * k : ioff + 256 * (k + 1),
                                    ],
                                    oa[u][0:DH, 256 * k : 256 * (k + 1)],
                                    rb[:, 256 * k : 256 * (k + 1)],
                                )
                        else:
                            nc.vector.tensor_mul(
                                attT[64 * u : 64 * u + 64, g, ioff : ioff + 512],
                                oa[u][0:DH, :],
                                rb[:],
                            )

                return norm

            def proj_unit(tt, half, tail=False):
                ps = ps_w.tile([128, 512], F32, tag="w", name=f"p{tt}_{half}")
                for kc2 in range(2):
                    nc.tensor.matmul(
                        ps[:],
                        attT[:, kc2, tt * 128 : (tt + 1) * 128],
                        wp_sb[:, kc2, half * 512 : (half + 1) * 512],
                        start=(kc2 == 0),
                        stop=(kc2 == 1),
                    )
                osb = po.tile([128, 512], BF16, tag="osb")
                if tail:
                    nc.scalar.copy(osb[:], ps[:])  # ACT is idle at the tail
                else:
                    nc.vector.tensor_copy(osb[:], ps[:])
                nc.sync.dma_start(
                    out_d.ap()[tt * 128 : (tt + 1) * 128, half * 512 : (half + 1) * 512],
                    osb[:],
                )

            # ---- weave: attention paced, producer/proj as PE filler;
            # qk units for the next head pair are the in-block fillers;
            # norm closures emitted one block late ----
            # (fillers must not feed the enclosing block's own scores(0):
            # att(bi,0) uses ct0/ct2 only, so qk(1,bi) is safe; att(bi,1)
            # for bi>=1 only needs qk(3,bi)'s tokens from step 4bi >= 4;
            # att(0,1) would deadlock on qk(3,0), so it gets v_unit(4).)
            qk_unit(0, 0)
            qk_unit(2, 0)
            for tt in range(4):
                v_unit(tt)
            P = lambda tt, half: (lambda: proj_unit(tt, half))
            n0 = att_block(0, 0, fillers={1: [lambda: qk_unit(1, 0)]})
            nc.sync.dma_start(xT_sb[:, 1], xT_ap[:, 1])
            qk_unit(3, 0)
            n0()
            n1 = att_block(0, 1, fillers={1: [lambda: v_unit(4)]})
            # bi = 1
            nc.sync.dma_start(xT_sb[:, 2], xT_ap[:, 2])
            nc.scalar.dma_start(wp_sb[:], wp_d.ap())
            qk_unit(0, 1)
            qk_unit(2, 1)
            n1()
            for tt in range(5, 8):
                v_unit(tt)
            for tt in range(0, 4):
                proj_unit(tt, 0)
                proj_unit(tt, 1)
            n0 = att_block(1, 0, fillers={1: [lambda: qk_unit(1, 1)]})
            n0()
            n1 = att_block(1, 1, fillers={1: [lambda: qk_unit(3, 1)]})
            # bi = 2
            nc.sync.dma_start(xT_sb[:, 3], xT_ap[:, 3])
            qk_unit(0, 2)
            qk_unit(2, 2)
            n1()
            for tt in range(8, 12):
                v_unit(tt)
            for tt in (4, 5, 7):
                proj_unit(tt, 0)
                proj_unit(tt, 1)
            n0 = att_block(2, 0, fillers={1: [lambda: qk_unit(1, 2)], 7: [P(6, 0)]})
            n0()
            n1 = att_block(2, 1, fillers={1: [lambda: qk_unit(3, 2)], 7: [P(6, 1)]})
            # bi = 3
            qk_unit(0, 3)
            qk_unit(2, 3)
            n1()
            for tt in range(12, 16):
                v_unit(tt)
            for tt in (8, 9):
                proj_unit(tt, 0)
                proj_unit(tt, 1)
            n0 = att_block(
                3, 0, fillers={1: [lambda: qk_unit(1, 3)], 7: [P(10, 0)], 12: [P(10, 1)]}
            )
            n0()
            n1 = att_block(3, 1, fillers={1: [lambda: qk_unit(3, 3)], 7: [P(11, 0)]})
            proj_unit(11, 1)
            n1()
            for tt in range(12, 16):
                proj_unit(tt, 0, tail=True)
                proj_unit(tt, 1, tail=True)

    nc.compile()
    return nc


def make_core_inputs(x, w_qkv, b_qkv, w_proj, b_proj):
    """Per-core input maps (host-side sharding)."""
    bf = mybir.dt.np(BF16)
    x = np.asarray(x, dtype=np.float32)
    w_qkv = np.asarray(w_qkv, dtype=np.float32)
    b_qkv = np.asarray(b_qkv, dtype=np.float32)
    w_proj = np.asarray(w_proj, dtype=np.float32)

    consts = {
        "triu": np.triu(np.ones((128, 128), dtype=np.float32)).astype(bf),
        "ones64": np.ones((128, 64), dtype=np.float32).astype(bf),
    }
    in_maps = []
    for c in range(8):
        b = c // 4
        heads = [4 * (c % 4) + i for i in range(HPC)]
        qcols = np.concatenate([np.arange(64 * h, 64 * h + 64) for h in heads])
        wq = w_qkv[:, qcols] * 0.125
        bq = b_qkv[qcols] * 0.125
        wk = w_qkv[:, C + qcols]
        bk = b_qkv[C + qcols]
        wv = w_qkv[:, 2 * C + qcols]
        wqk = np.concatenate([wq, wk], axis=1)          # [C, 512]
        # wqk4[ct, p, kc, m] = wqk[kc*128+p, ct*128+m]
        wqk4 = np.ascontiguousarray(
            wqk.reshape(KC, 128, 4, 128).transpose(2, 1, 0, 3)
        ).astype(bf)
        bqk = np.concatenate([bq, bk]).reshape(4, 128).T.copy()
        # wv4[p, kc, m] = wv[kc*128+p, m]
        wv4 = np.ascontiguousarray(
            wv.reshape(KC, 128, VCOLS).transpose(1, 0, 2)
        ).astype(bf)
        # wp4[p, kc2, n] = w_proj[qcols[kc2*128+p], n]
        wp4 = np.ascontiguousarray(
            w_proj[qcols, :].reshape(2, 128, C).transpose(1, 0, 2)
        ).astype(bf)
        # xT6[tb, p, kc, t] = x[b][tb*512+t, kc*128+p]
        xT6 = np.ascontiguousarray(
            x[b].reshape(NB, 512, KC, 128).transpose(0, 3, 2, 1)
        ).astype(bf)
        in_maps.append({
            "xT": xT6,
            "wqk4": wqk4,
            "bqk": bqk,
            "wv4": wv4,
            "wp4": wp4,
            **consts,
        })
    return in_maps


_NC_CACHE = []


def kernel(x, w_qkv, b_qkv, w_proj, b_proj):
    if not _NC_CACHE:
        _NC_CACHE.append(build_nc())
    nc = _NC_CACHE[0]
    in_maps = make_core_inputs(x, w_qkv, b_qkv, w_proj, b_proj)
    res = run_bass_kernel_spmd(nc, in_maps, list(range(8)))
    w_proj = np.asarray(w_proj, dtype=np.float64)
    bv = np.asarray(b_qkv, dtype=np.float64)[2 * C : 3 * C]
    b_eff = (np.asarray(b_proj, dtype=np.float64) + bv @ w_proj).astype(np.float32)
    out = np.empty((B, T, C), dtype=np.float32)
    for b in range(B):
        acc = res.results[4 * b]["out"].astype(np.float32)
        for c in range(4 * b + 1, 4 * b + 4):
            acc = acc + res.results[c]["out"].astype(np.float32)
        out[b] = acc + b_eff
    return out
